# revision 1
# baseline (speedup 1.0000x reference)
"""Bahdanau additive attention kernel for Trainium2 (8 NeuronCores).

Computes softmax_T(tanh(enc @ W1 + dec @ W2) @ V) for
enc [32, 4096, 512], dec [32, 512], W1/W2 [512, 512], V [512, 1].

Sharding: data-parallel over batch, 4 batches per core; W1/W2/V replicated.
enc is pre-cast to fp16 on the host (halves HBM+interconnect traffic; device
matmuls are fp16 anyway). Per-core pipeline: DMA enc tile -> transpose to
[F, T] layout (PE identity-matmul, with 2/8 of tiles routed through the DMA
xbar transpose to offload the LDWEIGHTS-bound PE) -> fp16 matmul vs W1 (fp32
PSUM) -> tanh(psum + W2^T dec bias) on ScalarE -> V-reduction matmul on PE ->
per-batch softmax (max/exp/sum/scale, fp32) -> DMA out.
Measured ~155-170 us on 8 axon-attached TRN2 cores (PE-bound; DMA ~80 us,
ACT ~110 us, DVE ~75 us busy). Pool depths matter: transpose-PSUM bufs=3 and tanh bufs=4 (at 2 the PE
stalls waiting on evacuation slots / V-reduce reads, ~+25 us).
"""

import numpy as np

B, T, F, H = 32, 4096, 512, 512
N_CORES = 8
B_LOCAL = B // N_CORES

_compiled = {}
ENC_NP_DTYPE = np.float16   # enc is pre-cast on host; device matmuls are fp16


def _build_program(T_tile=512, repeats=1, xbar_eighths=2, gpsimd_cast=False,
                   gpsimd_softmax=False, enc_swdge=False, nbufs=4, warmup=True,
                   xbar_burst=True, mm_bufs=3, sc_bufs=2, vr_fp8=False,
                   prefetch=2, enc_ring="sync", enc_f16_in=True,
                   xbar_ring="sync", evac_all_dve=False, tp_bufs=3,
                   tanh_bufs=4, sco_bufs=2):
    import concourse.bass as bass
    import concourse.mybir as mybir
    from concourse.tile import TileContext
    from concourse.masks import make_identity

    f32 = mybir.dt.float32
    f16 = mybir.dt.float16
    f8 = mybir.dt.float8e4
    AF = mybir.ActivationFunctionType
    ALU = mybir.AluOpType
    AX = mybir.AxisListType
    tanh_dt = f8 if vr_fp8 else f16

    S = T_tile // 128          # 128-row sub-blocks per T tile
    NT = T // T_tile           # T tiles per batch
    KC = F // 128              # contraction chunks
    HC = H // 128              # H chunks
    TS = 512                   # matmul free-dim (one PSUM bank)
    NH = T_tile // TS          # TS-halves per T tile

    nc = bass.Bass("TRN2", target_bir_lowering=False, debug=False,
                   num_devices=N_CORES)

    enc = nc.dram_tensor("encoder_outputs", [B_LOCAL, T, F],
                         f16 if enc_f16_in else f32,
                         kind="ExternalInput").ap()
    dec = nc.dram_tensor("dec_output", [B_LOCAL, F], f32,
                         kind="ExternalInput").ap()
    W1d = nc.dram_tensor("W1", [F, H], f32, kind="ExternalInput").ap()
    W2d = nc.dram_tensor("W2", [F, H], f32, kind="ExternalInput").ap()
    Vd = nc.dram_tensor("V", [H, 1], f32, kind="ExternalInput").ap()
    out = nc.dram_tensor("out", [B_LOCAL, T], f32, kind="ExternalOutput").ap()

    def enc_dma(enc_nat, b, tt):
        eng = {"sync": nc.sync, "scalar": nc.scalar,
               "gpsimd": nc.gpsimd}["gpsimd" if enc_swdge else enc_ring]
        eng.dma_start(
            enc_nat[:],
            enc[b, tt * T_tile:(tt + 1) * T_tile, :]
            .rearrange("(s p) f -> p s f", p=128))

    with TileContext(nc) as tc:
        with tc.tile_pool(name="consts", bufs=1) as consts, \
             tc.tile_pool(name="scores", bufs=sco_bufs) as scores_pool, \
             tc.tile_pool(name="probs", bufs=sco_bufs) as probs_pool, \
             tc.tile_pool(name="encnat", bufs=nbufs) as encnat_pool, \
             tc.tile_pool(name="small", bufs=1) as small:

            # issue the first enc loads before the setup DMAs so the main
            # pipeline's head isn't queued behind W1/W2 on the DMA ring
            prefetched = {}
            for u in range(min(prefetch, nbufs) if repeats == 1 else 0):
                if enc_f16_in:
                    t_pf = encnat_pool.tile([128, S, F], f16, tag="en")
                else:
                    t_pf = encnat_pool.tile([128, S, F], f32, tag="en")
                enc_dma(t_pf, u // NT, u % NT)
                prefetched[u] = t_pf

            # ---- constants / setup ----
            idn16 = consts.tile([128, 128], f16)
            make_identity(nc, idn16[:])
            idn32 = consts.tile([128, 128], f32)
            make_identity(nc, idn32[:])

            w1_32 = small.tile([128, KC, H], f32)
            nc.sync.dma_start(w1_32[:], W1d.rearrange("(k p) h -> p k h", p=128))
            w1_16 = consts.tile([128, KC, H], f16)
            nc.vector.tensor_copy(w1_16[:], w1_32[:])

            v_sb = small.tile([128, HC], f32)
            for k in range(HC):
                nc.sync.dma_start(v_sb[:, k:k + 1], Vd[k * 128:(k + 1) * 128, :])
            v16 = consts.tile([128, HC], f16)
            nc.vector.tensor_copy(v16[:], v_sb[:])
            if vr_fp8:
                # [Ki, 2, M] interleaved weight pairs for DoubleRow; padded
                # M stride to keep the Ko step 16B-aligned
                v8 = consts.tile([128, HC // 2, 2, 16], f8)
                nc.vector.memset(v8[:], 0.0)
                for i in range(HC // 2):
                    for j in range(2):
                        nc.vector.tensor_copy(v8[:, i, j, 0:1],
                                              v_sb[:, 2 * i + j:2 * i + j + 1])

            # w2T[h, b] = sum_f W2[f, h] * dec[b, f], kept fp32 as tanh bias
            w2_32 = small.tile([128, KC, H], f32)
            nc.sync.dma_start(w2_32[:], W2d.rearrange("(k p) h -> p k h", p=128))
            dec_pad = small.tile([128, F], f32)
            nc.vector.memset(dec_pad[:], 0.0)
            nc.sync.dma_start(dec_pad[:B_LOCAL, :], dec[:, :])
            decT = small.tile([128, KC, B_LOCAL], f32)
            w2T = consts.tile([128, HC, B_LOCAL], f32)
            with tc.tile_pool(name="setup_ps", bufs=2, space="PSUM") as sps:
                for k in range(KC):
                    tp = sps.tile([128, 128], f32, tag="dec_tp")
                    nc.tensor.transpose(tp[:], dec_pad[:, k * 128:(k + 1) * 128],
                                        idn32[:])
                    nc.vector.tensor_copy(decT[:, k, :], tp[:, :B_LOCAL])
                for hc in range(HC):
                    pw = sps.tile([128, B_LOCAL], f32, tag="w2_ps")
                    for k in range(KC):
                        nc.tensor.matmul(pw[:], w2_32[:, k, hc * 128:(hc + 1) * 128],
                                         decT[:, k, :], start=(k == 0),
                                         stop=(k == KC - 1))
                    nc.vector.tensor_copy(w2T[:, hc, :], pw[:])

            # ---- main pipeline ----
            with tc.tile_pool(name="enc16", bufs=nbufs) as enc16_pool, \
                 tc.tile_pool(name="encT", bufs=nbufs) as encT_pool, \
                 tc.tile_pool(name="tanh", bufs=tanh_bufs) as tanh_pool, \
                 tc.tile_pool(name="tp_ps", bufs=tp_bufs, space="PSUM") as tp_psum, \
                 tc.tile_pool(name="mm_ps", bufs=mm_bufs, space="PSUM") as mm_psum, \
                 tc.tile_pool(name="sc_ps", bufs=sc_bufs, space="PSUM") as sc_psum:

                # HAM warmup: a short burst of matmuls while the first enc
                # tile streams in, so real matmuls start at 2.4 GHz
                if warmup:
                    wps = mm_psum.tile([128, TS], f32, tag="mm")
                    for i in range(24):
                        nc.tensor.matmul(wps[:], idn16[:],
                                         w1_16[:, i % KC, :],
                                         start=(i == 0), stop=(i == 23))

                for b in [bb for _ in range(repeats) for bb in range(B_LOCAL)]:
                    scores_b = scores_pool.tile([1, NT, NH, TS], f32, tag="sc")
                    for tt in range(NT):
                        uidx = b * NT + tt
                        if uidx in prefetched and repeats == 1:
                            enc_nat = prefetched.pop(uidx)
                        else:
                            enc_nat = encnat_pool.tile(
                                [128, S, F], f16 if enc_f16_in else f32,
                                tag="en")
                            enc_dma(enc_nat, b, tt)
                        if enc_f16_in:
                            enc16 = enc_nat
                        else:
                            enc16 = enc16_pool.tile([128, S, F], f16,
                                                    tag="e16")
                            cast_eng = (nc.gpsimd if gpsimd_cast
                                        else nc.vector)
                            cast_eng.tensor_copy(enc16[:], enc_nat[:])

                        encT = encT_pool.tile([128, KC, T_tile], f16, tag="eT")
                        tanh_sb = tanh_pool.tile([128, HC, NH, TS], tanh_dt,
                                                 tag="th")
                        for h in range(NH):
                            # Route a fraction of transposes via the DMA xbar
                            # to offload the PE (LDWEIGHTS-bound transposes).
                            half_idx = (b * NT + tt) * NH + h
                            if half_idx % 8 < xbar_eighths:
                                xeng = (nc.scalar if xbar_ring == "scalar"
                                        else nc.sync)
                                for s4 in range(4):
                                    sa = h * 4 + s4
                                    xeng.dma_start_transpose(
                                        encT[:, :, sa * 128:(sa + 1) * 128],
                                        enc16[:, sa, :])
                            else:
                                for k in range(KC):
                                    tp = tp_psum.tile([128, 512], f16, tag="tp")
                                    for s in range(4):
                                        nc.tensor.transpose(
                                            tp[:, s * 128:(s + 1) * 128],
                                            enc16[:, h * 4 + s,
                                                  k * 128:(k + 1) * 128],
                                            idn16[:])
                                    eng = (nc.vector if (k < 3 or evac_all_dve)
                                           else nc.scalar)
                                    if eng is nc.vector:
                                        eng.tensor_copy(
                                            encT[:, k, h * TS:(h + 1) * TS],
                                            tp[:])
                                    else:
                                        nc.scalar.copy(
                                            encT[:, k, h * TS:(h + 1) * TS],
                                            tp[:])
                            for hc in range(HC):
                                mm = mm_psum.tile([128, TS], f32, tag="mm")
                                for k in range(KC):
                                    nc.tensor.matmul(
                                        mm[:],
                                        w1_16[:, k, hc * 128:(hc + 1) * 128],
                                        encT[:, k, h * TS:(h + 1) * TS],
                                        start=(k == 0), stop=(k == KC - 1))
                                nc.scalar.activation(
                                    tanh_sb[:, hc, h, :], mm[:], AF.Tanh,
                                    bias=w2T[:, hc, b:b + 1])
                            sc = sc_psum.tile([1, TS], f32, tag="sc_ps")
                            if vr_fp8:
                                for i in range(HC // 2):
                                    nc.tensor.matmul(
                                        sc[:], v8[:, i, :, 0:1],
                                        tanh_sb[:, 2 * i:2 * i + 2, h, :],
                                        start=(i == 0), stop=(i == HC // 2 - 1),
                                        perf_mode=mybir.MatmulPerfMode.DoubleRow)
                            else:
                                for hc in range(HC):
                                    nc.tensor.matmul(
                                        sc[:], v16[:, hc:hc + 1],
                                        tanh_sb[:, hc, h, :],
                                        start=(hc == 0), stop=(hc == HC - 1))
                            nc.vector.tensor_copy(scores_b[:, tt, h, :], sc[:])

                    # ---- softmax over T for this batch ----
                    mx = scores_pool.tile([1, 1], f32, tag="mx")
                    if gpsimd_softmax:
                        nc.gpsimd.tensor_reduce(mx[:], scores_b[:], AX.XYZWC,
                                                ALU.max)
                    else:
                        nc.vector.tensor_reduce(mx[:], scores_b[:], AX.XYZ,
                                                ALU.max)
                    nc.vector.tensor_scalar_mul(mx[:], mx[:], -1.0)
                    probs_t = probs_pool.tile([1, NT, NH, TS], f32, tag="pb")
                    den = scores_pool.tile([1, 1], f32, tag="den")
                    nc.scalar.activation(probs_t[:], scores_b[:], AF.Exp,
                                         bias=mx[:], accum_out=den[:])
                    rden = scores_pool.tile([1, 1], f32, tag="rden")
                    nc.vector.reciprocal(rden[:], den[:])
                    scale_eng = nc.gpsimd if gpsimd_softmax else nc.vector
                    scale_eng.tensor_scalar_mul(probs_t[:], probs_t[:], rden[:])
                    nc.sync.dma_start(
                        out[b:b + 1, :].rearrange("o (x y z) -> o x y z",
                                                  x=NT, y=NH, z=TS),
                        probs_t[:])

    _split_multi_waits(nc)
    return nc


def _split_multi_waits(nc):
    """Walrus CTRL-type lowering only accepts one sync-wait per instruction;
    hoist extra waits onto same-engine NoOps inserted right before."""
    import concourse.mybir as mybir
    for fn in nc.m.functions:
        for blk in fn.blocks:
            new = []
            for inst in blk.instructions:
                si = getattr(inst, "sync_info", None)
                if si is not None and si.on_wait and len(si.on_wait) > 1:
                    waits = list(si.on_wait)
                    for w in waits[:-1]:
                        nop = mybir.InstNoOp(
                            name=nc.get_next_instruction_name(),
                            engine=inst.engine, ins=[], outs=[],
                            sync_info=mybir.SyncInfo(on_wait=[w], on_update=[]))
                        new.append(nop)
                    inst.sync_info = mybir.SyncInfo(
                        on_wait=[waits[-1]], on_update=list(si.on_update))
                new.append(inst)
            blk.instructions[:] = new


def _make_runner(nc):
    """Build a cached shard_map-jitted executor over the 8 NeuronCores
    (mirrors concourse.bass2jax.run_bass_via_pjrt, but reusable across
    calls so repeat invocations skip retracing)."""
    import jax
    from jax.sharding import Mesh, PartitionSpec, NamedSharding
    from jax.experimental.shard_map import shard_map
    import concourse.mybir as mybir
    from concourse import bass2jax
    from concourse.bass2jax import _bass_exec_p, install_neuronx_cc_hook

    install_neuronx_cc_hook()
    partition_name = (nc.partition_id_tensor.name
                      if nc.partition_id_tensor else None)
    in_names, out_names, out_avals, zero_outs = [], [], [], []
    for alloc in nc.m.functions[0].allocations:
        if not isinstance(alloc, mybir.MemoryLocationSet):
            continue
        name = alloc.memorylocations[0].name
        if alloc.kind == "ExternalInput":
            if name != partition_name:
                in_names.append(name)
        elif alloc.kind == "ExternalOutput":
            out_names.append(name)
            out_avals.append(jax.core.ShapedArray(
                tuple(alloc.tensor_shape), mybir.dt.np(alloc.dtype)))
            zero_outs.append(np.zeros(tuple(alloc.tensor_shape),
                                      mybir.dt.np(alloc.dtype)))
    n_params = len(in_names)
    n_outs = len(out_avals)
    all_names = list(in_names) + list(out_names)
    if partition_name is not None:
        all_names.append(partition_name)

    def _body(*args):
        operands = list(args)
        if partition_name is not None:
            operands.append(bass2jax.partition_id_tensor())
        outs = _bass_exec_p.bind(
            *operands,
            out_avals=tuple(out_avals),
            in_names=tuple(all_names),
            out_names=tuple(out_names),
            lowering_input_output_aliases=(),
            sim_require_finite=True,
            sim_require_nnan=True,
            nc=nc)
        return tuple(outs)

    devices = jax.devices()[:N_CORES]
    assert len(devices) == N_CORES, f"need {N_CORES} cores, saw {devices}"
    mesh = Mesh(np.asarray(devices), ("core",))
    fn = jax.jit(
        shard_map(_body, mesh=mesh,
                  in_specs=(PartitionSpec("core"),) * (n_params + n_outs),
                  out_specs=(PartitionSpec("core"),) * n_outs,
                  check_rep=False),
        donate_argnums=tuple(range(n_params, n_params + n_outs)),
        keep_unused=True)
    shard = NamedSharding(mesh, PartitionSpec("core"))
    return fn, in_names, out_names, zero_outs, shard


def kernel(encoder_outputs, dec_output, W1, W2, V):
    import jax

    if "runner" not in _compiled:
        _compiled["runner"] = _make_runner(_build_program())
    fn, in_names, out_names, zero_outs, shard = _compiled["runner"]

    full = {
        "encoder_outputs": np.ascontiguousarray(encoder_outputs,
                                                dtype=ENC_NP_DTYPE),
        "dec_output": np.ascontiguousarray(dec_output, dtype=np.float32),
        "W1": np.ascontiguousarray(W1, dtype=np.float32),
        "W2": np.ascontiguousarray(W2, dtype=np.float32),
        "V": np.ascontiguousarray(V, dtype=np.float32),
    }

    def core_slice(name, c):
        a = full[name]
        if name in ("encoder_outputs", "dec_output"):
            return a[c * B_LOCAL:(c + 1) * B_LOCAL]
        return a

    concat_in = [
        np.concatenate([core_slice(n, c) for c in range(N_CORES)], axis=0)
        for n in in_names
    ]
    dev_in = [jax.device_put(a, shard) for a in concat_in]
    dev_zeros = [
        jax.device_put(np.zeros((N_CORES * z.shape[0], *z.shape[1:]),
                                z.dtype), shard)
        for z in zero_outs
    ]
    outs = fn(*dev_in, *dev_zeros)
    out = np.asarray(outs[out_names.index("out")])
    return out.reshape(B, T)



# revision 3
# speedup vs baseline: 1.1354x; 1.1354x over previous
"""Bahdanau additive attention kernel for Trainium2 (8 NeuronCores).

Computes softmax_T(tanh(enc @ W1 + dec @ W2) @ V) for
enc [32, 4096, 512], dec [32, 512], W1/W2 [512, 512], V [512, 1].

Sharding: data-parallel over batch, 4 batches per core; W1/W2/V replicated.
enc is pre-cast to fp16 on the host (halves HBM+interconnect traffic; device
matmuls are fp16 anyway). Per-core pipeline: DMA enc tile -> transpose to
[F, T] layout (PE identity-matmul, with 2/8 of tiles routed through the DMA
xbar transpose to offload the LDWEIGHTS-bound PE) -> fp16 matmul vs W1 (fp32
PSUM) -> tanh(psum + W2^T dec bias) on ScalarE -> V-reduction matmul on PE ->
per-batch softmax (max/exp/sum/scale, fp32) -> DMA out.
Measured ~155-170 us on 8 axon-attached TRN2 cores (PE-bound; DMA ~80 us,
ACT ~110 us, DVE ~75 us busy). Pool depths matter: transpose-PSUM bufs=3 and tanh bufs=4 (at 2 the PE
stalls waiting on evacuation slots / V-reduce reads, ~+25 us).
"""

import numpy as np

B, T, F, H = 32, 4096, 512, 512
N_CORES = 8
B_LOCAL = B // N_CORES

_compiled = {}
ENC_NP_DTYPE = np.float16   # enc is pre-cast on host; device matmuls are fp16


def _build_program(T_tile=512, repeats=1, xbar_eighths=2, gpsimd_cast=False,
                   gpsimd_softmax=False, enc_swdge=False, nbufs=4, warmup=True,
                   xbar_burst=True, mm_bufs=3, sc_bufs=2, vr_fp8=False,
                   prefetch=2, enc_ring="sync", enc_f16_in=True,
                   xbar_ring="sync", evac_all_dve=False, tp_bufs=3,
                   tanh_bufs=4, sco_bufs=2, loop_repeats=1):
    import concourse.bass as bass
    import concourse.mybir as mybir
    from concourse.tile import TileContext
    from concourse.masks import make_identity

    f32 = mybir.dt.float32
    f16 = mybir.dt.float16
    f8 = mybir.dt.float8e4
    AF = mybir.ActivationFunctionType
    ALU = mybir.AluOpType
    AX = mybir.AxisListType
    tanh_dt = f8 if vr_fp8 else f16

    S = T_tile // 128          # 128-row sub-blocks per T tile
    NT = T // T_tile           # T tiles per batch
    KC = F // 128              # contraction chunks
    HC = H // 128              # H chunks
    TS = 512                   # matmul free-dim (one PSUM bank)
    NH = T_tile // TS          # TS-halves per T tile

    nc = bass.Bass("TRN2", target_bir_lowering=False, debug=False,
                   num_devices=N_CORES)

    enc = nc.dram_tensor("encoder_outputs", [B_LOCAL, T, F],
                         f16 if enc_f16_in else f32,
                         kind="ExternalInput").ap()
    dec = nc.dram_tensor("dec_output", [B_LOCAL, F], f32,
                         kind="ExternalInput").ap()
    W1d = nc.dram_tensor("W1", [F, H], f32, kind="ExternalInput").ap()
    W2d = nc.dram_tensor("W2", [F, H], f32, kind="ExternalInput").ap()
    Vd = nc.dram_tensor("V", [H, 1], f32, kind="ExternalInput").ap()
    out = nc.dram_tensor("out", [B_LOCAL, T], f32, kind="ExternalOutput").ap()

    def enc_dma(enc_nat, b, tt):
        eng = {"sync": nc.sync, "scalar": nc.scalar,
               "gpsimd": nc.gpsimd}["gpsimd" if enc_swdge else enc_ring]
        eng.dma_start(
            enc_nat[:],
            enc[b, tt * T_tile:(tt + 1) * T_tile, :]
            .rearrange("(s p) f -> p s f", p=128))

    with TileContext(nc) as tc:
        with tc.tile_pool(name="consts", bufs=1) as consts, \
             tc.tile_pool(name="scores", bufs=sco_bufs) as scores_pool, \
             tc.tile_pool(name="probs", bufs=sco_bufs) as probs_pool, \
             tc.tile_pool(name="encnat", bufs=nbufs) as encnat_pool, \
             tc.tile_pool(name="small", bufs=1) as small:

            # issue the first enc loads before the setup DMAs so the main
            # pipeline's head isn't queued behind W1/W2 on the DMA ring
            prefetched = {}
            for u in range(min(prefetch, nbufs) if repeats == 1 else 0):
                if enc_f16_in:
                    t_pf = encnat_pool.tile([128, S, F], f16, tag="en")
                else:
                    t_pf = encnat_pool.tile([128, S, F], f32, tag="en")
                enc_dma(t_pf, u // NT, u % NT)
                prefetched[u] = t_pf

            # ---- constants / setup ----
            idn16 = consts.tile([128, 128], f16)
            make_identity(nc, idn16[:])
            idn32 = consts.tile([128, 128], f32)
            make_identity(nc, idn32[:])

            w1_32 = small.tile([128, KC, H], f32)
            nc.sync.dma_start(w1_32[:], W1d.rearrange("(k p) h -> p k h", p=128))
            w1_16 = consts.tile([128, KC, H], f16)
            nc.vector.tensor_copy(w1_16[:], w1_32[:])

            v_sb = small.tile([128, HC], f32)
            for k in range(HC):
                nc.sync.dma_start(v_sb[:, k:k + 1], Vd[k * 128:(k + 1) * 128, :])
            v16 = consts.tile([128, HC], f16)
            nc.vector.tensor_copy(v16[:], v_sb[:])
            if vr_fp8:
                # [Ki, 2, M] interleaved weight pairs for DoubleRow; padded
                # M stride to keep the Ko step 16B-aligned
                v8 = consts.tile([128, HC // 2, 2, 16], f8)
                nc.vector.memset(v8[:], 0.0)
                for i in range(HC // 2):
                    for j in range(2):
                        nc.vector.tensor_copy(v8[:, i, j, 0:1],
                                              v_sb[:, 2 * i + j:2 * i + j + 1])

            # w2T[h, b] = sum_f W2[f, h] * dec[b, f], kept fp32 as tanh bias
            w2_32 = small.tile([128, KC, H], f32)
            nc.sync.dma_start(w2_32[:], W2d.rearrange("(k p) h -> p k h", p=128))
            dec_pad = small.tile([128, F], f32)
            nc.vector.memset(dec_pad[:], 0.0)
            nc.sync.dma_start(dec_pad[:B_LOCAL, :], dec[:, :])
            decT = small.tile([128, KC, B_LOCAL], f32)
            w2T = consts.tile([128, HC, B_LOCAL], f32)
            with tc.tile_pool(name="setup_ps", bufs=2, space="PSUM") as sps:
                for k in range(KC):
                    tp = sps.tile([128, 128], f32, tag="dec_tp")
                    nc.tensor.transpose(tp[:], dec_pad[:, k * 128:(k + 1) * 128],
                                        idn32[:])
                    nc.vector.tensor_copy(decT[:, k, :], tp[:, :B_LOCAL])
                for hc in range(HC):
                    pw = sps.tile([128, B_LOCAL], f32, tag="w2_ps")
                    for k in range(KC):
                        nc.tensor.matmul(pw[:], w2_32[:, k, hc * 128:(hc + 1) * 128],
                                         decT[:, k, :], start=(k == 0),
                                         stop=(k == KC - 1))
                    nc.vector.tensor_copy(w2T[:, hc, :], pw[:])

            # ---- main pipeline ----
            with tc.tile_pool(name="enc16", bufs=nbufs) as enc16_pool, \
                 tc.tile_pool(name="encT", bufs=nbufs) as encT_pool, \
                 tc.tile_pool(name="tanh", bufs=tanh_bufs) as tanh_pool, \
                 tc.tile_pool(name="tp_ps", bufs=tp_bufs, space="PSUM") as tp_psum, \
                 tc.tile_pool(name="mm_ps", bufs=mm_bufs, space="PSUM") as mm_psum, \
                 tc.tile_pool(name="sc_ps", bufs=sc_bufs, space="PSUM") as sc_psum:

                # HAM warmup: a short burst of matmuls while the first enc
                # tile streams in, so real matmuls start at 2.4 GHz
                if warmup:
                    wps = mm_psum.tile([128, TS], f32, tag="mm")
                    for i in range(24):
                        nc.tensor.matmul(wps[:], idn16[:],
                                         w1_16[:, i % KC, :],
                                         start=(i == 0), stop=(i == 23))

                import contextlib
                loop_ctx = (tc.For_i(0, loop_repeats) if loop_repeats > 1
                            else contextlib.nullcontext())
                with loop_ctx:
                  for b in [bb for _ in range(repeats) for bb in range(B_LOCAL)]:
                    scores_b = scores_pool.tile([1, NT, NH, TS], f32, tag="sc")
                    for tt in range(NT):
                        uidx = b * NT + tt
                        if uidx in prefetched and repeats == 1:
                            enc_nat = prefetched.pop(uidx)
                        else:
                            enc_nat = encnat_pool.tile(
                                [128, S, F], f16 if enc_f16_in else f32,
                                tag="en")
                            enc_dma(enc_nat, b, tt)
                        if enc_f16_in:
                            enc16 = enc_nat
                        else:
                            enc16 = enc16_pool.tile([128, S, F], f16,
                                                    tag="e16")
                            cast_eng = (nc.gpsimd if gpsimd_cast
                                        else nc.vector)
                            cast_eng.tensor_copy(enc16[:], enc_nat[:])

                        encT = encT_pool.tile([128, KC, T_tile], f16, tag="eT")
                        tanh_sb = tanh_pool.tile([128, HC, NH, TS], tanh_dt,
                                                 tag="th")
                        for h in range(NH):
                            # Route a fraction of transposes via the DMA xbar
                            # to offload the PE (LDWEIGHTS-bound transposes).
                            half_idx = (b * NT + tt) * NH + h
                            if half_idx % 8 < xbar_eighths:
                                xeng = (nc.scalar if xbar_ring == "scalar"
                                        else nc.sync)
                                for s4 in range(4):
                                    sa = h * 4 + s4
                                    xeng.dma_start_transpose(
                                        encT[:, :, sa * 128:(sa + 1) * 128],
                                        enc16[:, sa, :])
                            else:
                                for k in range(KC):
                                    tp = tp_psum.tile([128, 512], f16, tag="tp")
                                    for s in range(4):
                                        nc.tensor.transpose(
                                            tp[:, s * 128:(s + 1) * 128],
                                            enc16[:, h * 4 + s,
                                                  k * 128:(k + 1) * 128],
                                            idn16[:])
                                    eng = (nc.vector if (k < 3 or evac_all_dve)
                                           else nc.scalar)
                                    if eng is nc.vector:
                                        eng.tensor_copy(
                                            encT[:, k, h * TS:(h + 1) * TS],
                                            tp[:])
                                    else:
                                        nc.scalar.copy(
                                            encT[:, k, h * TS:(h + 1) * TS],
                                            tp[:])
                            for hc in range(HC):
                                mm = mm_psum.tile([128, TS], f32, tag="mm")
                                for k in range(KC):
                                    nc.tensor.matmul(
                                        mm[:],
                                        w1_16[:, k, hc * 128:(hc + 1) * 128],
                                        encT[:, k, h * TS:(h + 1) * TS],
                                        start=(k == 0), stop=(k == KC - 1))
                                nc.scalar.activation(
                                    tanh_sb[:, hc, h, :], mm[:], AF.Tanh,
                                    bias=w2T[:, hc, b:b + 1])
                            sc = sc_psum.tile([1, TS], f32, tag="sc_ps")
                            if vr_fp8:
                                for i in range(HC // 2):
                                    nc.tensor.matmul(
                                        sc[:], v8[:, i, :, 0:1],
                                        tanh_sb[:, 2 * i:2 * i + 2, h, :],
                                        start=(i == 0), stop=(i == HC // 2 - 1),
                                        perf_mode=mybir.MatmulPerfMode.DoubleRow)
                            else:
                                for hc in range(HC):
                                    nc.tensor.matmul(
                                        sc[:], v16[:, hc:hc + 1],
                                        tanh_sb[:, hc, h, :],
                                        start=(hc == 0), stop=(hc == HC - 1))
                            nc.vector.tensor_copy(scores_b[:, tt, h, :], sc[:])

                    # ---- softmax over T for this batch ----
                    mx = scores_pool.tile([1, 1], f32, tag="mx")
                    if gpsimd_softmax:
                        nc.gpsimd.tensor_reduce(mx[:], scores_b[:], AX.XYZWC,
                                                ALU.max)
                    else:
                        nc.vector.tensor_reduce(mx[:], scores_b[:], AX.XYZ,
                                                ALU.max)
                    nc.vector.tensor_scalar_mul(mx[:], mx[:], -1.0)
                    probs_t = probs_pool.tile([1, NT, NH, TS], f32, tag="pb")
                    den = scores_pool.tile([1, 1], f32, tag="den")
                    nc.scalar.activation(probs_t[:], scores_b[:], AF.Exp,
                                         bias=mx[:], accum_out=den[:])
                    rden = scores_pool.tile([1, 1], f32, tag="rden")
                    nc.vector.reciprocal(rden[:], den[:])
                    scale_eng = nc.gpsimd if gpsimd_softmax else nc.vector
                    scale_eng.tensor_scalar_mul(probs_t[:], probs_t[:], rden[:])
                    nc.sync.dma_start(
                        out[b:b + 1, :].rearrange("o (x y z) -> o x y z",
                                                  x=NT, y=NH, z=TS),
                        probs_t[:])

    _split_multi_waits(nc)
    return nc


def _split_multi_waits(nc):
    """Walrus CTRL-type lowering only accepts one sync-wait per instruction;
    hoist extra waits onto same-engine NoOps inserted right before."""
    import concourse.mybir as mybir
    for fn in nc.m.functions:
        for blk in fn.blocks:
            new = []
            for inst in blk.instructions:
                si = getattr(inst, "sync_info", None)
                if si is not None and si.on_wait and len(si.on_wait) > 1:
                    waits = list(si.on_wait)
                    for w in waits[:-1]:
                        nop = mybir.InstNoOp(
                            name=nc.get_next_instruction_name(),
                            engine=inst.engine, ins=[], outs=[],
                            sync_info=mybir.SyncInfo(on_wait=[w], on_update=[]))
                        new.append(nop)
                    inst.sync_info = mybir.SyncInfo(
                        on_wait=[waits[-1]], on_update=list(si.on_update))
                new.append(inst)
            blk.instructions[:] = new


def _make_runner(nc):
    """Build a cached shard_map-jitted executor over the 8 NeuronCores
    (mirrors concourse.bass2jax.run_bass_via_pjrt, but reusable across
    calls so repeat invocations skip retracing)."""
    import jax
    from jax.sharding import Mesh, PartitionSpec, NamedSharding
    from jax.experimental.shard_map import shard_map
    import concourse.mybir as mybir
    from concourse import bass2jax
    from concourse.bass2jax import _bass_exec_p, install_neuronx_cc_hook

    install_neuronx_cc_hook()
    partition_name = (nc.partition_id_tensor.name
                      if nc.partition_id_tensor else None)
    in_names, out_names, out_avals, zero_outs = [], [], [], []
    for alloc in nc.m.functions[0].allocations:
        if not isinstance(alloc, mybir.MemoryLocationSet):
            continue
        name = alloc.memorylocations[0].name
        if alloc.kind == "ExternalInput":
            if name != partition_name:
                in_names.append(name)
        elif alloc.kind == "ExternalOutput":
            out_names.append(name)
            out_avals.append(jax.core.ShapedArray(
                tuple(alloc.tensor_shape), mybir.dt.np(alloc.dtype)))
            zero_outs.append(np.zeros(tuple(alloc.tensor_shape),
                                      mybir.dt.np(alloc.dtype)))
    n_params = len(in_names)
    n_outs = len(out_avals)
    all_names = list(in_names) + list(out_names)
    if partition_name is not None:
        all_names.append(partition_name)

    def _body(*args):
        operands = list(args)
        if partition_name is not None:
            operands.append(bass2jax.partition_id_tensor())
        outs = _bass_exec_p.bind(
            *operands,
            out_avals=tuple(out_avals),
            in_names=tuple(all_names),
            out_names=tuple(out_names),
            lowering_input_output_aliases=(),
            sim_require_finite=True,
            sim_require_nnan=True,
            nc=nc)
        return tuple(outs)

    devices = jax.devices()[:N_CORES]
    assert len(devices) == N_CORES, f"need {N_CORES} cores, saw {devices}"
    mesh = Mesh(np.asarray(devices), ("core",))
    fn = jax.jit(
        shard_map(_body, mesh=mesh,
                  in_specs=(PartitionSpec("core"),) * (n_params + n_outs),
                  out_specs=(PartitionSpec("core"),) * n_outs,
                  check_rep=False),
        donate_argnums=tuple(range(n_params, n_params + n_outs)),
        keep_unused=True)
    shard = NamedSharding(mesh, PartitionSpec("core"))
    return fn, in_names, out_names, zero_outs, shard


def kernel(encoder_outputs, dec_output, W1, W2, V):
    import jax

    if "runner" not in _compiled:
        _compiled["runner"] = _make_runner(_build_program())
    fn, in_names, out_names, zero_outs, shard = _compiled["runner"]

    full = {
        "encoder_outputs": np.ascontiguousarray(encoder_outputs,
                                                dtype=ENC_NP_DTYPE),
        "dec_output": np.ascontiguousarray(dec_output, dtype=np.float32),
        "W1": np.ascontiguousarray(W1, dtype=np.float32),
        "W2": np.ascontiguousarray(W2, dtype=np.float32),
        "V": np.ascontiguousarray(V, dtype=np.float32),
    }

    def core_slice(name, c):
        a = full[name]
        if name in ("encoder_outputs", "dec_output"):
            return a[c * B_LOCAL:(c + 1) * B_LOCAL]
        return a

    concat_in = [
        np.concatenate([core_slice(n, c) for c in range(N_CORES)], axis=0)
        for n in in_names
    ]
    dev_in = [jax.device_put(a, shard) for a in concat_in]
    dev_zeros = [
        jax.device_put(np.zeros((N_CORES * z.shape[0], *z.shape[1:]),
                                z.dtype), shard)
        for z in zero_outs
    ]
    outs = fn(*dev_in, *dev_zeros)
    out = np.asarray(outs[out_names.index("out")])
    return out.reshape(B, T)



# revision 10
# speedup vs baseline: 1.5046x; 1.3251x over previous
"""Bahdanau additive attention kernel for Trainium2 (8 NeuronCores).

Computes softmax_T(tanh(enc @ W1 + dec @ W2) @ V) for
enc [32, 4096, 512], dec [32, 512], W1/W2 [512, 512], V [512, 1].

Sharding: data-parallel over batch, 4 batches per core; W1/W2/V replicated.
enc is pre-cast to fp16 on the host (halves HBM+interconnect traffic; device
matmuls are fp16 anyway).

Per-core pipeline (v2):
 - enc tiles are loaded DIRECTLY transposed from DRAM via the DMA xbar
   transpose (dma_start_transpose [512,512] -> [128,KC,512]), which fuses
   the HBM read and the [T,F]->[F,T] transpose into one DMA; the PE no
   longer does any transposes and there is no separate enc load.
 - fp16 matmul vs W1 (fp32 PSUM), tanh(psum + W2^T dec bias) on ScalarE.
 - V-reduction: vfold="dve": DVE folds the HC chunks (z = sum_hc V_hc*tanh_hc,
   per-partition tensor_scalar muls + adds), then 4 tiny PE matmuls
   (z chunk stationary x ones) write scores TRANSPOSED [128(t%128), tt, ch]
   into a PSUM bank; vfold="pe": classic V-stationary matmul [1,512] +
   DVE evac + 4 tiny PE row->column matmuls into the same layout.
 - softmax over T per batch on the [128, NT*4] transposed layout: ACT exp
   with per-partition accum, PE sums/broadcasts via 1-col matmuls, DVE
   scales; no max subtraction (scores are O(5), fp32 exp is exact enough).
   Output transposed back via one PE transpose -> contiguous DMA out.
"""

import numpy as np

B, T, F, H = 32, 4096, 512, 512
N_CORES = 8
B_LOCAL = B // N_CORES

_compiled = {}
ENC_NP_DTYPE = np.float16   # enc is pre-cast on host; device matmuls are fp16


def _build_program(T_tile=512, repeats=1, loop_repeats=1, vfold="dve",
                   mm_bufs=4, sc_bufs=2, sp_bufs=2, encT_bufs=4,
                   tanh_bufs=4, z_bufs=3, e_bufs=2, scv_bufs=2,
                   warmup=True, prefetch=2, split_waits=True):
    import concourse.bass as bass
    import concourse.mybir as mybir
    from concourse.tile import TileContext
    from concourse.masks import make_identity
    import contextlib

    f32 = mybir.dt.float32
    f16 = mybir.dt.float16
    AF = mybir.ActivationFunctionType
    ALU = mybir.AluOpType

    NT = T // T_tile           # T tiles per batch
    KC = F // 128              # contraction chunks
    HC = H // 128              # H chunks
    TS = T_tile                # matmul free-dim (one PSUM bank, <=512)
    CH = T_tile // 128         # 128-column chunks per tile (scores transposed)
    NCOL = NT * CH             # score columns per batch in [128, NCOL] layout
    assert TS <= 512 and NCOL * 4 <= 2048  # scores bank: NCOL f32 <= 2KB

    if vfold == "pe":
        mm_bufs = min(mm_bufs, 3)
        sp_bufs = min(sp_bufs, 1)
        psum_budget = mm_bufs + sc_bufs + sp_bufs + scv_bufs
    else:
        psum_budget = mm_bufs + sc_bufs + sp_bufs
    assert psum_budget <= 8, f"PSUM over budget: {psum_budget}"

    nc = bass.Bass("TRN2", target_bir_lowering=False, debug=False,
                   num_devices=N_CORES)

    enc = nc.dram_tensor("encoder_outputs", [B_LOCAL, T, F], f16,
                         kind="ExternalInput").ap()
    dec = nc.dram_tensor("dec_output", [B_LOCAL, F], f32,
                         kind="ExternalInput").ap()
    W1d = nc.dram_tensor("W1", [F, H], f32, kind="ExternalInput").ap()
    W2d = nc.dram_tensor("W2", [F, H], f32, kind="ExternalInput").ap()
    Vd = nc.dram_tensor("V", [H, 1], f32, kind="ExternalInput").ap()
    out = nc.dram_tensor("out", [B_LOCAL, T], f32, kind="ExternalOutput").ap()

    def enc_dma_t(encT_tile, b, tt):
        # DRAM [T_tile, F] --xbar--> SBUF [128, KC, T_tile], f = k*128 + p
        nc.sync.dma_start_transpose(
            encT_tile[:], enc[b, tt * T_tile:(tt + 1) * T_tile, :])

    with TileContext(nc) as tc:
        with tc.tile_pool(name="consts", bufs=1) as consts, \
             tc.tile_pool(name="encT", bufs=encT_bufs) as encT_pool, \
             tc.tile_pool(name="small", bufs=1) as small:

            # issue the first enc transpose-loads before the setup DMAs so
            # the pipeline head isn't queued behind W1/W2 on the DMA ring
            prefetched = {}
            for u in range(prefetch if (repeats == 1 and loop_repeats == 1)
                           else 0):
                t_pf = encT_pool.tile([128, KC, T_tile], f16, tag="eT")
                enc_dma_t(t_pf, u // NT, u % NT)
                prefetched[u] = t_pf

            # ---- constants / setup ----
            idn32 = consts.tile([128, 128], f32)
            make_identity(nc, idn32[:])
            ones_col32 = consts.tile([128, 1], f32)
            nc.vector.memset(ones_col32[:], 1.0)
            ones_row32 = consts.tile([1, 128], f32)
            nc.vector.memset(ones_row32[:], 1.0)
            ones16 = consts.tile([128, 1], f16)
            nc.vector.memset(ones16[:], 1.0)

            w1_32 = small.tile([128, KC, H], f32)
            nc.sync.dma_start(w1_32[:], W1d.rearrange("(k p) h -> p k h", p=128))
            w1_16 = consts.tile([128, KC, H], f16)
            nc.vector.tensor_copy(w1_16[:], w1_32[:])

            v_sb = small.tile([128, HC], f32)
            for k in range(HC):
                nc.sync.dma_start(v_sb[:, k:k + 1], Vd[k * 128:(k + 1) * 128, :])
            v16 = consts.tile([128, HC], f16)
            nc.vector.tensor_copy(v16[:], v_sb[:])
            v32 = consts.tile([128, HC], f32)
            nc.vector.tensor_copy(v32[:], v_sb[:])

            # w2T[h, b] = sum_f W2[f, h] * dec[b, f], kept fp32 as tanh bias
            w2_32 = small.tile([128, KC, H], f32)
            nc.sync.dma_start(w2_32[:], W2d.rearrange("(k p) h -> p k h", p=128))
            dec_pad = small.tile([128, F], f32)
            nc.vector.memset(dec_pad[:], 0.0)
            nc.sync.dma_start(dec_pad[:B_LOCAL, :], dec[:, :])
            decT = small.tile([128, KC, B_LOCAL], f32)
            w2T = consts.tile([128, HC, B_LOCAL], f32)
            with tc.tile_pool(name="setup_ps", bufs=2, space="PSUM") as sps:
                for k in range(KC):
                    tp = sps.tile([128, 128], f32, tag="dec_tp")
                    nc.tensor.transpose(tp[:], dec_pad[:, k * 128:(k + 1) * 128],
                                        idn32[:])
                    nc.vector.tensor_copy(decT[:, k, :], tp[:, :B_LOCAL])
                for hc in range(HC):
                    pw = sps.tile([128, B_LOCAL], f32, tag="w2_ps")
                    for k in range(KC):
                        nc.tensor.matmul(pw[:], w2_32[:, k, hc * 128:(hc + 1) * 128],
                                         decT[:, k, :], start=(k == 0),
                                         stop=(k == KC - 1))
                    nc.vector.tensor_copy(w2T[:, hc, :], pw[:])

            # ---- main pipeline ----
            with tc.tile_pool(name="tanh", bufs=tanh_bufs) as tanh_pool, \
                 tc.tile_pool(name="z", bufs=z_bufs) as z_pool, \
                 tc.tile_pool(name="svec", bufs=scv_bufs) as svec_pool, \
                 tc.tile_pool(name="e", bufs=e_bufs) as e_pool, \
                 tc.tile_pool(name="smx", bufs=2) as smx_pool, \
                 tc.tile_pool(name="mm_ps", bufs=mm_bufs, space="PSUM") as mm_psum, \
                 tc.tile_pool(name="sc_ps", bufs=sc_bufs, space="PSUM") as sc_psum, \
                 tc.tile_pool(name="sp_ps", bufs=sp_bufs, space="PSUM") as sp_psum, \
                 (tc.tile_pool(name="scv_ps", bufs=scv_bufs, space="PSUM")
                  if vfold == "pe" else contextlib.nullcontext()) as scv_psum:

                # HAM warmup: a short burst of matmuls while the first enc
                # tile streams in, so real matmuls start at 2.4 GHz
                if warmup:
                    wps = mm_psum.tile([128, TS], f32, tag="mm")
                    for i in range(24):
                        nc.tensor.matmul(wps[:], w1_16[:, i % KC, :128],
                                         w1_16[:, (i + 1) % KC, :TS],
                                         start=(i == 0), stop=(i == 23))

                loop_ctx = (tc.For_i(0, loop_repeats) if loop_repeats > 1
                            else contextlib.nullcontext())
                with loop_ctx:
                  for b in [bb for _ in range(repeats) for bb in range(B_LOCAL)]:
                    # transposed scores for this batch: [t%128, tt, ch]
                    sc_b = sc_psum.tile([128, NT, CH], f32, tag="scb")
                    for tt in range(NT):
                        uidx = b * NT + tt
                        if uidx in prefetched:
                            encT = prefetched.pop(uidx)
                        else:
                            encT = encT_pool.tile([128, KC, T_tile], f16,
                                                  tag="eT")
                            enc_dma_t(encT, b, tt)

                        tanh_sb = tanh_pool.tile([128, HC, TS], f16, tag="th")
                        for hc in range(HC):
                            mm = mm_psum.tile([128, TS], f32, tag="mm")
                            for k in range(KC):
                                nc.tensor.matmul(
                                    mm[:],
                                    w1_16[:, k, hc * 128:(hc + 1) * 128],
                                    encT[:, k, :],
                                    start=(k == 0), stop=(k == KC - 1))
                            nc.scalar.activation(
                                tanh_sb[:, hc, :], mm[:], AF.Tanh,
                                bias=w2T[:, hc, b:b + 1])

                        if vfold == "dve":
                            # z[p,t] = sum_hc v16[hc*128+p] * tanh[p,hc,t]
                            z = z_pool.tile([128, HC, TS], f16, tag="z")
                            for hc in range(HC):
                                nc.vector.tensor_scalar_mul(
                                    z[:, hc, :], tanh_sb[:, hc, :],
                                    v32[:, hc:hc + 1])
                            nc.vector.tensor_add(z[:, 0, :], z[:, 0, :],
                                                 z[:, 1, :])
                            nc.vector.tensor_add(z[:, 2, :], z[:, 2, :],
                                                 z[:, 3, :])
                            nc.vector.tensor_add(z[:, 0, :], z[:, 0, :],
                                                 z[:, 2, :])
                            for ch in range(CH):
                                nc.tensor.matmul(
                                    sc_b[:, tt, ch:ch + 1],
                                    z[:, 0, ch * 128:(ch + 1) * 128],
                                    ones16[:],
                                    start=True, stop=True)
                        else:
                            sc = scv_psum.tile([1, TS], f32, tag="sc_ps")
                            for hc in range(HC):
                                nc.tensor.matmul(
                                    sc[:], v16[:, hc:hc + 1],
                                    tanh_sb[:, hc, :],
                                    start=(hc == 0), stop=(hc == HC - 1))
                            svec = svec_pool.tile([1, TS], f32, tag="sv")
                            nc.vector.tensor_copy(svec[:], sc[:])
                            # scatter the row into transposed score columns
                            for ch in range(CH):
                                nc.tensor.matmul(
                                    sc_b[:, tt, ch:ch + 1],
                                    svec[:, ch * 128:(ch + 1) * 128],
                                    ones_row32[:, 0:1],
                                    start=True, stop=True)

                    # ---- softmax over T for this batch (transposed layout) --
                    e = e_pool.tile([128, NT, CH], f32, tag="e")
                    acc = smx_pool.tile([128, 1], f32, tag="acc")
                    nc.scalar.activation(e[:], sc_b[:], AF.Exp,
                                         accum_out=acc[:])
                    Dp = sp_psum.tile([1, 1], f32, tag="sp")
                    nc.tensor.matmul(Dp[:], acc[:], ones_col32[:],
                                     start=True, stop=True)
                    Ds = smx_pool.tile([1, 1], f32, tag="Ds")
                    nc.vector.tensor_copy(Ds[:], Dp[:])
                    r = smx_pool.tile([1, 1], f32, tag="r")
                    nc.vector.reciprocal(r[:], Ds[:])
                    rbp = sp_psum.tile([128, 1], f32, tag="sp")
                    nc.tensor.matmul(rbp[:], ones_row32[:], r[:],
                                     start=True, stop=True)
                    rbs = smx_pool.tile([128, 1], f32, tag="rbs")
                    nc.vector.tensor_copy(rbs[:], rbp[:])
                    probs = e_pool.tile([128, NT, CH], f32, tag="pb")
                    nc.vector.tensor_scalar_mul(probs[:], e[:], rbs[:, 0:1])
                    pTp = sp_psum.tile([NCOL, 128], f32, tag="sp")
                    nc.tensor.transpose(pTp[:], probs[:], idn32[:])
                    pTs = smx_pool.tile([NCOL, 128], f32, tag="pTs")
                    nc.vector.tensor_copy(pTs[:], pTp[:])
                    nc.sync.dma_start(
                        out[b:b + 1, :].rearrange("o (c p) -> (o c) p",
                                                  c=NCOL),
                        pTs[:])

    if split_waits:
        _split_multi_waits(nc)
    return nc


def _split_multi_waits(nc):
    """Walrus CTRL-type lowering only accepts one sync-wait per instruction;
    hoist extra waits onto same-engine NoOps inserted right before."""
    import concourse.mybir as mybir
    for fn in nc.m.functions:
        for blk in fn.blocks:
            new = []
            for inst in blk.instructions:
                si = getattr(inst, "sync_info", None)
                if si is not None and si.on_wait and len(si.on_wait) > 1:
                    waits = list(si.on_wait)
                    for w in waits[:-1]:
                        nop = mybir.InstNoOp(
                            name=nc.get_next_instruction_name(),
                            engine=inst.engine, ins=[], outs=[],
                            sync_info=mybir.SyncInfo(on_wait=[w], on_update=[]))
                        new.append(nop)
                    inst.sync_info = mybir.SyncInfo(
                        on_wait=[waits[-1]], on_update=list(si.on_update))
                new.append(inst)
            blk.instructions[:] = new


def _make_runner(nc):
    """Build a cached shard_map-jitted executor over the 8 NeuronCores
    (mirrors concourse.bass2jax.run_bass_via_pjrt, but reusable across
    calls so repeat invocations skip retracing)."""
    import jax
    from jax.sharding import Mesh, PartitionSpec, NamedSharding
    from jax.experimental.shard_map import shard_map
    import concourse.mybir as mybir
    from concourse import bass2jax
    from concourse.bass2jax import _bass_exec_p, install_neuronx_cc_hook

    install_neuronx_cc_hook()
    partition_name = (nc.partition_id_tensor.name
                      if nc.partition_id_tensor else None)
    in_names, out_names, out_avals, zero_outs = [], [], [], []
    for alloc in nc.m.functions[0].allocations:
        if not isinstance(alloc, mybir.MemoryLocationSet):
            continue
        name = alloc.memorylocations[0].name
        if alloc.kind == "ExternalInput":
            if name != partition_name:
                in_names.append(name)
        elif alloc.kind == "ExternalOutput":
            out_names.append(name)
            out_avals.append(jax.core.ShapedArray(
                tuple(alloc.tensor_shape), mybir.dt.np(alloc.dtype)))
            zero_outs.append(np.zeros(tuple(alloc.tensor_shape),
                                      mybir.dt.np(alloc.dtype)))
    n_params = len(in_names)
    n_outs = len(out_avals)
    all_names = list(in_names) + list(out_names)
    if partition_name is not None:
        all_names.append(partition_name)

    def _body(*args):
        operands = list(args)
        if partition_name is not None:
            operands.append(bass2jax.partition_id_tensor())
        outs = _bass_exec_p.bind(
            *operands,
            out_avals=tuple(out_avals),
            in_names=tuple(all_names),
            out_names=tuple(out_names),
            lowering_input_output_aliases=(),
            sim_require_finite=True,
            sim_require_nnan=True,
            nc=nc)
        return tuple(outs)

    devices = jax.devices()[:N_CORES]
    assert len(devices) == N_CORES, f"need {N_CORES} cores, saw {devices}"
    mesh = Mesh(np.asarray(devices), ("core",))
    fn = jax.jit(
        shard_map(_body, mesh=mesh,
                  in_specs=(PartitionSpec("core"),) * (n_params + n_outs),
                  out_specs=(PartitionSpec("core"),) * n_outs,
                  check_rep=False),
        donate_argnums=tuple(range(n_params, n_params + n_outs)),
        keep_unused=True)
    shard = NamedSharding(mesh, PartitionSpec("core"))
    return fn, in_names, out_names, zero_outs, shard


def kernel(encoder_outputs, dec_output, W1, W2, V):
    import jax

    if "runner" not in _compiled:
        _compiled["runner"] = _make_runner(_build_program())
    fn, in_names, out_names, zero_outs, shard = _compiled["runner"]

    full = {
        "encoder_outputs": np.ascontiguousarray(encoder_outputs,
                                                dtype=ENC_NP_DTYPE),
        "dec_output": np.ascontiguousarray(dec_output, dtype=np.float32),
        "W1": np.ascontiguousarray(W1, dtype=np.float32),
        "W2": np.ascontiguousarray(W2, dtype=np.float32),
        "V": np.ascontiguousarray(V, dtype=np.float32),
    }

    def core_slice(name, c):
        a = full[name]
        if name in ("encoder_outputs", "dec_output"):
            return a[c * B_LOCAL:(c + 1) * B_LOCAL]
        return a

    concat_in = [
        np.concatenate([core_slice(n, c) for c in range(N_CORES)], axis=0)
        for n in in_names
    ]
    dev_in = [jax.device_put(a, shard) for a in concat_in]
    dev_zeros = [
        jax.device_put(np.zeros((N_CORES * z.shape[0], *z.shape[1:]),
                                z.dtype), shard)
        for z in zero_outs
    ]
    outs = fn(*dev_in, *dev_zeros)
    out = np.asarray(outs[out_names.index("out")])
    return out.reshape(B, T)


# revision 22
# speedup vs baseline: 1.5906x; 1.0571x over previous
"""Bahdanau additive attention kernel for Trainium2 (8 NeuronCores).

Computes softmax_T(tanh(enc @ W1 + dec @ W2) @ V) for
enc [32, 4096, 512], dec [32, 512], W1/W2 [512, 512], V [512, 1].

Sharding: data-parallel over batch, 4 batches per core; W1/W2/V replicated.
enc is pre-cast to fp16 on the host (halves HBM+interconnect traffic; device
matmuls are fp16 anyway).

Per-core pipeline (v2):
 - enc tiles are loaded DIRECTLY transposed from DRAM via the DMA xbar
   transpose (dma_start_transpose [512,512] -> [128,KC,512]), which fuses
   the HBM read and the [T,F]->[F,T] transpose into one DMA; the PE no
   longer does any transposes and there is no separate enc load.
 - fp16 matmul vs W1 (fp32 PSUM), tanh(psum + W2^T dec bias) on ScalarE.
 - V-reduction: vfold="dve": DVE folds the HC chunks (z = sum_hc V_hc*tanh_hc,
   per-partition tensor_scalar muls + adds), then 4 tiny PE matmuls
   (z chunk stationary x ones) write scores TRANSPOSED [128(t%128), tt, ch]
   into a PSUM bank; vfold="pe": classic V-stationary matmul [1,512] +
   DVE evac + 4 tiny PE row->column matmuls into the same layout.
 - softmax over T per batch on the [128, NT*4] transposed layout: ACT exp
   with per-partition accum, PE sums/broadcasts via 1-col matmuls, DVE
   scales; no max subtraction (scores are O(5), fp32 exp is exact enough).
   Output transposed back via one PE transpose -> contiguous DMA out.
"""

import numpy as np

B, T, F, H = 32, 4096, 512, 512
N_CORES = 8
B_LOCAL = B // N_CORES

_compiled = {}
ENC_NP_DTYPE = np.float16   # enc is pre-cast on host; device matmuls are fp16


def _build_program(T_tile=512, repeats=1, loop_repeats=1, vfold="dve",
                   mm_bufs=4, sc_bufs=2, sp_bufs=2, encT_bufs=4,
                   tanh_bufs=4, z_bufs=3, e_bufs=2, scv_bufs=2,
                   warmup=True, prefetch=2, split_waits=True,
                   ablate=None, ring2=False, zdma=0, tail="lite"):
    import concourse.bass as bass
    import concourse.mybir as mybir
    from concourse.tile import TileContext
    from concourse.masks import make_identity
    import contextlib

    f32 = mybir.dt.float32
    f16 = mybir.dt.float16
    AF = mybir.ActivationFunctionType
    ALU = mybir.AluOpType

    NT = T // T_tile           # T tiles per batch
    KC = F // 128              # contraction chunks
    HC = H // 128              # H chunks
    TS = T_tile                # matmul free-dim (one PSUM bank, <=512)
    CH = T_tile // 128         # 128-column chunks per tile (scores transposed)
    NCOL = NT * CH             # score columns per batch in [128, NCOL] layout
    assert TS <= 512 and NCOL * 4 <= 2048  # scores bank: NCOL f32 <= 2KB

    if vfold == "pe":
        mm_bufs = min(mm_bufs, 3)
        sp_bufs = min(sp_bufs, 1)
        psum_budget = mm_bufs + sc_bufs + sp_bufs + scv_bufs
    else:
        psum_budget = mm_bufs + sc_bufs + sp_bufs
    assert psum_budget <= 8, f"PSUM over budget: {psum_budget}"

    nc = bass.Bass("TRN2", target_bir_lowering=False, debug=False,
                   num_devices=N_CORES)

    enc = nc.dram_tensor("encoder_outputs", [B_LOCAL, T, F], f16,
                         kind="ExternalInput").ap()
    dec = nc.dram_tensor("dec_output", [B_LOCAL, F], f32,
                         kind="ExternalInput").ap()
    W1d = nc.dram_tensor("W1", [F, H], f32, kind="ExternalInput").ap()
    W2d = nc.dram_tensor("W2", [F, H], f32, kind="ExternalInput").ap()
    Vd = nc.dram_tensor("V", [H, 1], f32, kind="ExternalInput").ap()
    out = nc.dram_tensor("out", [B_LOCAL, T], f32, kind="ExternalOutput").ap()

    def enc_dma_t(encT_tile, b, tt):
        # DRAM [T_tile, F] --xbar--> SBUF [128, KC, T_tile], f = k*128 + p
        eng = nc.sync
        if ring2 and (b * NT + tt) % 2 == 1:
            eng = nc.scalar
        if ablate == "dbldma":
            eng.dma_start_transpose(
                encT_tile[:], enc[b, tt * T_tile:(tt + 1) * T_tile, :])
        eng.dma_start_transpose(
            encT_tile[:], enc[b, tt * T_tile:(tt + 1) * T_tile, :])

    with TileContext(nc) as tc:
        with tc.tile_pool(name="consts", bufs=1) as consts, \
             tc.tile_pool(name="encT", bufs=encT_bufs) as encT_pool, \
             tc.tile_pool(name="small", bufs=1) as small:

            # issue the first enc transpose-loads before the setup DMAs so
            # the pipeline head isn't queued behind W1/W2 on the DMA ring
            prefetched = {}
            for u in range(prefetch if (repeats == 1 and loop_repeats == 1)
                           else 0):
                t_pf = encT_pool.tile([128, KC, T_tile], f16, tag="eT")
                enc_dma_t(t_pf, u // NT, u % NT)
                prefetched[u] = t_pf

            # ---- constants / setup ----
            idn32 = consts.tile([128, 128], f32)
            make_identity(nc, idn32[:])
            ones_col32 = consts.tile([128, 1], f32)
            nc.vector.memset(ones_col32[:], 1.0)
            ones_row32 = consts.tile([1, 128], f32)
            nc.vector.memset(ones_row32[:], 1.0)
            ones16 = consts.tile([128, 1], f16)
            nc.vector.memset(ones16[:], 1.0)
            if vfold == "gps":
                from concourse import library_config
                nc.gpsimd.load_library(library_config.attn)

            w1_32 = small.tile([128, KC, H], f32)
            nc.sync.dma_start(w1_32[:], W1d.rearrange("(k p) h -> p k h", p=128))
            w1_16 = consts.tile([128, KC, H], f16)
            nc.vector.tensor_copy(w1_16[:], w1_32[:])

            v_sb = small.tile([128, HC], f32)
            for k in range(HC):
                nc.sync.dma_start(v_sb[:, k:k + 1], Vd[k * 128:(k + 1) * 128, :])
            v16 = consts.tile([128, HC], f16)
            nc.vector.tensor_copy(v16[:], v_sb[:])
            v32 = consts.tile([128, HC], f32)
            nc.vector.tensor_copy(v32[:], v_sb[:])

            # w2T[h, b] = sum_f W2[f, h] * dec[b, f], kept fp32 as tanh bias
            w2_32 = small.tile([128, KC, H], f32)
            nc.sync.dma_start(w2_32[:], W2d.rearrange("(k p) h -> p k h", p=128))
            dec_pad = small.tile([128, F], f32)
            nc.vector.memset(dec_pad[:], 0.0)
            nc.sync.dma_start(dec_pad[:B_LOCAL, :], dec[:, :])
            decT = small.tile([128, KC, B_LOCAL], f32)
            w2T = consts.tile([128, HC, B_LOCAL], f32)
            with tc.tile_pool(name="setup_ps", bufs=2, space="PSUM") as sps:
                for k in range(KC):
                    tp = sps.tile([128, 128], f32, tag="dec_tp")
                    nc.tensor.transpose(tp[:], dec_pad[:, k * 128:(k + 1) * 128],
                                        idn32[:])
                    nc.vector.tensor_copy(decT[:, k, :], tp[:, :B_LOCAL])
                for hc in range(HC):
                    pw = sps.tile([128, B_LOCAL], f32, tag="w2_ps")
                    for k in range(KC):
                        nc.tensor.matmul(pw[:], w2_32[:, k, hc * 128:(hc + 1) * 128],
                                         decT[:, k, :], start=(k == 0),
                                         stop=(k == KC - 1))
                    nc.vector.tensor_copy(w2T[:, hc, :], pw[:])

            # ---- main pipeline ----
            with tc.tile_pool(name="tanh", bufs=tanh_bufs) as tanh_pool, \
                 tc.tile_pool(name="z", bufs=z_bufs) as z_pool, \
                 tc.tile_pool(name="svec", bufs=scv_bufs) as svec_pool, \
                 tc.tile_pool(name="e", bufs=e_bufs) as e_pool, \
                 tc.tile_pool(name="smx", bufs=2) as smx_pool, \
                 tc.tile_pool(name="mm_ps", bufs=mm_bufs, space="PSUM") as mm_psum, \
                 tc.tile_pool(name="sc_ps", bufs=sc_bufs, space="PSUM") as sc_psum, \
                 tc.tile_pool(name="sp_ps", bufs=sp_bufs, space="PSUM") as sp_psum, \
                 (tc.tile_pool(name="scv_ps", bufs=scv_bufs, space="PSUM")
                  if vfold == "pe" else contextlib.nullcontext()) as scv_psum:

                # HAM warmup: a short burst of matmuls while the first enc
                # tile streams in, so real matmuls start at 2.4 GHz
                if warmup:
                    wps = mm_psum.tile([128, TS], f32, tag="mm")
                    for i in range(24):
                        nc.tensor.matmul(wps[:], w1_16[:, i % KC, :128],
                                         w1_16[:, (i + 1) % KC, :TS],
                                         start=(i == 0), stop=(i == 23))

                def emit_tail(b, sc_b):
                    e = e_pool.tile([128, NT, CH], f32, tag="e")
                    acc = smx_pool.tile([128, 1], f32, tag="acc")
                    nc.scalar.activation(e[:], sc_b[:], AF.Exp,
                                         accum_out=acc[:])
                    Dp = sp_psum.tile([1, 1], f32, tag="sp")
                    nc.tensor.matmul(Dp[:], acc[:], ones_col32[:],
                                     start=True, stop=True)
                    Ds = smx_pool.tile([1, 1], f32, tag="Ds")
                    nc.vector.tensor_copy(Ds[:], Dp[:])
                    r = smx_pool.tile([1, 1], f32, tag="r")
                    nc.vector.reciprocal(r[:], Ds[:])
                    pTp = sp_psum.tile([NCOL, 128], f32, tag="sp")
                    nc.tensor.transpose(pTp[:], e[:], idn32[:])
                    rbp = sp_psum.tile([NCOL, 1], f32, tag="sp")
                    nc.tensor.matmul(rbp[:], ones_row32[:, 0:NCOL],
                                     r[:], start=True, stop=True)
                    rbs = smx_pool.tile([NCOL, 1], f32, tag="rbs")
                    nc.vector.tensor_copy(rbs[:], rbp[:])
                    pTs = smx_pool.tile([NCOL, 128], f32, tag="pTs")
                    nc.vector.tensor_scalar_mul(pTs[:], pTp[:],
                                                rbs[:, 0:1])
                    nc.sync.dma_start(
                        out[b:b + 1, :].rearrange("o (c p) -> (o c) p",
                                                  c=NCOL),
                        pTs[:])

                def emit_pipelined(seq):
                    # tile i-1's 4 score-scatter matmuls are interleaved
                    # between tile i's 4 main-matmul groups so their
                    # LDWEIGHTS hide under the 512-cycle streams
                    pending = []
                    tail_due = None
                    sc_b = None
                    for i, (b, tt) in enumerate(seq):
                        if tt == 0:
                            sc_b = sc_psum.tile([128, NT, CH], f32,
                                                tag="scb")
                        cur_sc = sc_b
                        if i in prefetched:
                            encT = prefetched.pop(i)
                        else:
                            encT = encT_pool.tile([128, KC, T_tile], f16,
                                                  tag="eT")
                            enc_dma_t(encT, b, tt)
                        tanh_sb = tanh_pool.tile([128, HC, TS], f16,
                                                 tag="th")
                        for hc in range(HC):
                            mm = mm_psum.tile([128, TS], f32, tag="mm")
                            for k in range(KC):
                                nc.tensor.matmul(
                                    mm[:],
                                    w1_16[:, k, hc * 128:(hc + 1) * 128],
                                    encT[:, k, :],
                                    start=(k == 0), stop=(k == KC - 1))
                            nc.scalar.activation(
                                tanh_sb[:, hc, :], mm[:], AF.Tanh,
                                bias=w2T[:, hc, b:b + 1])
                            if pending:
                                pending.pop(0)()
                                if not pending and tail_due is not None:
                                    emit_tail(*tail_due)
                                    tail_due = None
                        z = z_pool.tile([128, HC, TS], f16, tag="z")
                        for hc in range(HC):
                            nc.vector.tensor_scalar_mul(
                                z[:, hc, :], tanh_sb[:, hc, :],
                                v32[:, hc:hc + 1])
                        nc.vector.tensor_add(z[:, 0, :], z[:, 0, :],
                                             z[:, 1, :])
                        nc.vector.tensor_add(z[:, 2, :], z[:, 2, :],
                                             z[:, 3, :])
                        nc.vector.tensor_add(z[:, 0, :], z[:, 0, :],
                                             z[:, 2, :])

                        def scat(ch, z=z, sc=cur_sc, tt=tt):
                            nc.tensor.matmul(
                                sc[:, tt, ch:ch + 1],
                                z[:, 0, ch * 128:(ch + 1) * 128],
                                ones16[:],
                                start=True, stop=True)
                        pending = [lambda ch=ch: scat(ch)
                                   for ch in range(CH)]
                        if tt == NT - 1:
                            tail_due = (b, cur_sc)
                    for f in pending:
                        f()
                    if tail_due is not None:
                        emit_tail(*tail_due)

                loop_ctx = (tc.For_i(0, loop_repeats) if loop_repeats > 1
                            else contextlib.nullcontext())
                with loop_ctx:
                  if vfold == "dvei":
                    emit_pipelined([(bb, tt)
                                    for _ in range(repeats)
                                    for bb in range(B_LOCAL)
                                    for tt in range(NT)])
                  else:
                   for b in [bb for _ in range(repeats) for bb in range(B_LOCAL)]:
                    # transposed scores for this batch: [t%128, tt, ch]
                    sc_b = sc_psum.tile([128, NT, CH], f32, tag="scb")
                    for tt in range(NT):
                        uidx = b * NT + tt
                        if uidx in prefetched:
                            encT = prefetched.pop(uidx)
                        else:
                            encT = encT_pool.tile([128, KC, T_tile], f16,
                                                  tag="eT")
                            if ablate != "nodma":
                                enc_dma_t(encT, b, tt)

                        tanh_sb = tanh_pool.tile([128, HC, TS], f16, tag="th")
                        for hc in range(HC):
                            mm = mm_psum.tile([128, TS], f32, tag="mm")
                            for k in range(KC):
                                nc.tensor.matmul(
                                    mm[:],
                                    w1_16[:, k, hc * 128:(hc + 1) * 128],
                                    encT[:, k, :],
                                    start=(k == 0), stop=(k == KC - 1))
                            if ablate == "dblact":
                                nc.scalar.activation(
                                    tanh_sb[:, hc, :], mm[:], AF.Tanh,
                                    bias=w2T[:, hc, b:b + 1])
                            nc.scalar.activation(
                                tanh_sb[:, hc, :], mm[:], AF.Tanh,
                                bias=w2T[:, hc, b:b + 1])

                        if ablate == "novred":
                            pass
                        elif vfold == "gps":
                            import concourse.bass_isa as bass_isa
                            z = z_pool.tile([128, HC, TS], f16, tag="z")
                            for hc in range(HC):
                                nc.vector.tensor_scalar_mul(
                                    z[:, hc, :], tanh_sb[:, hc, :],
                                    v32[:, hc:hc + 1])
                            nc.vector.tensor_add(z[:, 0, :], z[:, 0, :],
                                                 z[:, 1, :])
                            nc.vector.tensor_add(z[:, 2, :], z[:, 2, :],
                                                 z[:, 3, :])
                            nc.vector.tensor_add(z[:, 0, :], z[:, 0, :],
                                                 z[:, 2, :])
                            zr = z_pool.tile([128, TS], f32, tag="zr")
                            nc.gpsimd.partition_all_reduce(
                                zr[:], z[:, 0, :], 128,
                                bass_isa.ReduceOp.add)
                            for ch in range(CH):
                                nc.tensor.matmul(
                                    sc_b[:, tt, ch:ch + 1],
                                    zr[0:1, ch * 128:(ch + 1) * 128],
                                    ones_row32[:, 0:1],
                                    start=True, stop=True)
                        elif vfold in ("dve", "mixed"):
                            # z[p,t] = sum_hc v16[hc*128+p] * tanh[p,hc,t]
                            add_eng = (nc.gpsimd if vfold == "mixed"
                                       else nc.vector)
                            z = z_pool.tile([128, HC, TS], f16, tag="z")
                            if ablate == "dbldve":
                                for hc in range(HC):
                                    nc.vector.tensor_scalar_mul(
                                        z[:, hc, :], tanh_sb[:, hc, :],
                                        v32[:, hc:hc + 1])
                            if ablate != "nofold":
                                for hc in range(HC):
                                    nc.vector.tensor_scalar_mul(
                                        z[:, hc, :], tanh_sb[:, hc, :],
                                        v32[:, hc:hc + 1])
                                add_eng.tensor_add(z[:, 0, :], z[:, 0, :],
                                                   z[:, 1, :])
                                add_eng.tensor_add(z[:, 2, :], z[:, 2, :],
                                                   z[:, 3, :])
                                add_eng.tensor_add(z[:, 0, :], z[:, 0, :],
                                                   z[:, 2, :])
                            zsrc = (tanh_sb[:, 0, :] if ablate == "nofold"
                                    else z[:, 0, :])
                            kz = 128
                            while zdma > 0 and kz > zdma:
                                nc.gpsimd.dma_start(
                                    z[0:kz // 2, 0, :], z[kz // 2:kz, 0, :],
                                    accum_op=ALU.add)
                                kz //= 2
                            for ch in range(CH):
                                nc.tensor.matmul(
                                    sc_b[:, tt, ch:ch + 1],
                                    zsrc[0:kz, ch * 128:(ch + 1) * 128],
                                    ones16[0:kz, :],
                                    start=True, stop=True)
                        else:
                            sc = scv_psum.tile([1, TS], f32, tag="sc_ps")
                            for hc in range(HC):
                                nc.tensor.matmul(
                                    sc[:], v16[:, hc:hc + 1],
                                    tanh_sb[:, hc, :],
                                    start=(hc == 0), stop=(hc == HC - 1))
                            svec = svec_pool.tile([1, TS], f32, tag="sv")
                            nc.vector.tensor_copy(svec[:], sc[:])
                            # scatter the row into transposed score columns
                            for ch in range(CH):
                                nc.tensor.matmul(
                                    sc_b[:, tt, ch:ch + 1],
                                    svec[:, ch * 128:(ch + 1) * 128],
                                    ones_row32[:, 0:1],
                                    start=True, stop=True)

                    # ---- softmax over T for this batch (transposed layout) --
                    if ablate == "nosmx":
                        continue
                    e = e_pool.tile([128, NT, CH], f32, tag="e")
                    acc = smx_pool.tile([128, 1], f32, tag="acc")
                    nc.scalar.activation(e[:], sc_b[:], AF.Exp,
                                         accum_out=acc[:])
                    Dp = sp_psum.tile([1, 1], f32, tag="sp")
                    nc.tensor.matmul(Dp[:], acc[:], ones_col32[:],
                                     start=True, stop=True)
                    Ds = smx_pool.tile([1, 1], f32, tag="Ds")
                    nc.vector.tensor_copy(Ds[:], Dp[:])
                    r = smx_pool.tile([1, 1], f32, tag="r")
                    nc.vector.reciprocal(r[:], Ds[:])
                    if tail == "lite":
                        # transpose first; fuse the 1/D scale into the
                        # PSUM evacuation
                        pTp = sp_psum.tile([NCOL, 128], f32, tag="sp")
                        nc.tensor.transpose(pTp[:], e[:], idn32[:])
                        rbp = sp_psum.tile([NCOL, 1], f32, tag="sp")
                        nc.tensor.matmul(rbp[:], ones_row32[:, 0:NCOL],
                                         r[:], start=True, stop=True)
                        rbs = smx_pool.tile([NCOL, 1], f32, tag="rbs")
                        nc.vector.tensor_copy(rbs[:], rbp[:])
                        pTs = smx_pool.tile([NCOL, 128], f32, tag="pTs")
                        nc.vector.tensor_scalar_mul(pTs[:], pTp[:],
                                                    rbs[:, 0:1])
                    else:
                        rbp = sp_psum.tile([128, 1], f32, tag="sp")
                        nc.tensor.matmul(rbp[:], ones_row32[:], r[:],
                                         start=True, stop=True)
                        rbs = smx_pool.tile([128, 1], f32, tag="rbs")
                        nc.vector.tensor_copy(rbs[:], rbp[:])
                        probs = e_pool.tile([128, NT, CH], f32, tag="pb")
                        nc.vector.tensor_scalar_mul(probs[:], e[:],
                                                    rbs[:, 0:1])
                        pTp = sp_psum.tile([NCOL, 128], f32, tag="sp")
                        nc.tensor.transpose(pTp[:], probs[:], idn32[:])
                        pTs = smx_pool.tile([NCOL, 128], f32, tag="pTs")
                        nc.vector.tensor_copy(pTs[:], pTp[:])
                    nc.sync.dma_start(
                        out[b:b + 1, :].rearrange("o (c p) -> (o c) p",
                                                  c=NCOL),
                        pTs[:])

    if split_waits:
        _split_multi_waits(nc)
    return nc


def _split_multi_waits(nc):
    """Walrus CTRL-type lowering only accepts one sync-wait per instruction;
    hoist extra waits onto same-engine NoOps inserted right before."""
    import concourse.mybir as mybir
    for fn in nc.m.functions:
        for blk in fn.blocks:
            new = []
            for inst in blk.instructions:
                si = getattr(inst, "sync_info", None)
                if si is not None and si.on_wait and len(si.on_wait) > 1:
                    waits = list(si.on_wait)
                    for w in waits[:-1]:
                        nop = mybir.InstNoOp(
                            name=nc.get_next_instruction_name(),
                            engine=inst.engine, ins=[], outs=[],
                            sync_info=mybir.SyncInfo(on_wait=[w], on_update=[]))
                        new.append(nop)
                    inst.sync_info = mybir.SyncInfo(
                        on_wait=[waits[-1]], on_update=list(si.on_update))
                new.append(inst)
            blk.instructions[:] = new


def _make_runner(nc):
    """Build a cached shard_map-jitted executor over the 8 NeuronCores
    (mirrors concourse.bass2jax.run_bass_via_pjrt, but reusable across
    calls so repeat invocations skip retracing)."""
    import jax
    from jax.sharding import Mesh, PartitionSpec, NamedSharding
    from jax.experimental.shard_map import shard_map
    import concourse.mybir as mybir
    from concourse import bass2jax
    from concourse.bass2jax import _bass_exec_p, install_neuronx_cc_hook

    install_neuronx_cc_hook()
    partition_name = (nc.partition_id_tensor.name
                      if nc.partition_id_tensor else None)
    in_names, out_names, out_avals, zero_outs = [], [], [], []
    for alloc in nc.m.functions[0].allocations:
        if not isinstance(alloc, mybir.MemoryLocationSet):
            continue
        name = alloc.memorylocations[0].name
        if alloc.kind == "ExternalInput":
            if name != partition_name:
                in_names.append(name)
        elif alloc.kind == "ExternalOutput":
            out_names.append(name)
            out_avals.append(jax.core.ShapedArray(
                tuple(alloc.tensor_shape), mybir.dt.np(alloc.dtype)))
            zero_outs.append(np.zeros(tuple(alloc.tensor_shape),
                                      mybir.dt.np(alloc.dtype)))
    n_params = len(in_names)
    n_outs = len(out_avals)
    all_names = list(in_names) + list(out_names)
    if partition_name is not None:
        all_names.append(partition_name)

    def _body(*args):
        operands = list(args)
        if partition_name is not None:
            operands.append(bass2jax.partition_id_tensor())
        outs = _bass_exec_p.bind(
            *operands,
            out_avals=tuple(out_avals),
            in_names=tuple(all_names),
            out_names=tuple(out_names),
            lowering_input_output_aliases=(),
            sim_require_finite=True,
            sim_require_nnan=True,
            nc=nc)
        return tuple(outs)

    devices = jax.devices()[:N_CORES]
    assert len(devices) == N_CORES, f"need {N_CORES} cores, saw {devices}"
    mesh = Mesh(np.asarray(devices), ("core",))
    fn = jax.jit(
        shard_map(_body, mesh=mesh,
                  in_specs=(PartitionSpec("core"),) * (n_params + n_outs),
                  out_specs=(PartitionSpec("core"),) * n_outs,
                  check_rep=False),
        donate_argnums=tuple(range(n_params, n_params + n_outs)),
        keep_unused=True)
    shard = NamedSharding(mesh, PartitionSpec("core"))
    return fn, in_names, out_names, zero_outs, shard


def kernel(encoder_outputs, dec_output, W1, W2, V):
    import jax

    if "runner" not in _compiled:
        _compiled["runner"] = _make_runner(_build_program())
    fn, in_names, out_names, zero_outs, shard = _compiled["runner"]

    full = {
        "encoder_outputs": np.ascontiguousarray(encoder_outputs,
                                                dtype=ENC_NP_DTYPE),
        "dec_output": np.ascontiguousarray(dec_output, dtype=np.float32),
        "W1": np.ascontiguousarray(W1, dtype=np.float32),
        "W2": np.ascontiguousarray(W2, dtype=np.float32),
        "V": np.ascontiguousarray(V, dtype=np.float32),
    }

    def core_slice(name, c):
        a = full[name]
        if name in ("encoder_outputs", "dec_output"):
            return a[c * B_LOCAL:(c + 1) * B_LOCAL]
        return a

    concat_in = [
        np.concatenate([core_slice(n, c) for c in range(N_CORES)], axis=0)
        for n in in_names
    ]
    dev_in = [jax.device_put(a, shard) for a in concat_in]
    dev_zeros = [
        jax.device_put(np.zeros((N_CORES * z.shape[0], *z.shape[1:]),
                                z.dtype), shard)
        for z in zero_outs
    ]
    outs = fn(*dev_in, *dev_zeros)
    out = np.asarray(outs[out_names.index("out")])
    return out.reshape(B, T)
